# revision 22
# baseline (speedup 1.0000x reference)
"""Trainium2 Bass kernel for nn_ChebKernelMixture.

Computes gram(xs) = psi(xs) @ psi(xs).T where psi is a Chebyshev feature
map: psi(x) = concat_n sqrt(w_n) * phi_n(x), phi_0 = [1],
phi_n = [T_n(x), sqrt(1-x^2) U_{n-1}(x)], w = softmax(logits).

Shapes: xs (16384,), logits (33,) -> out (16384, 16384) f32.

Strategy (8 NeuronCores, SPMD, no collectives):
  - the host hands each core the full xs as a [128, 128] matrix whose
    rows are the 128 point-blocks in a core-specific order: position k
    holds global block (120 - 8*(k//8) + k%8 + c) mod 128.  The rotation
    by c puts the core's own row blocks at fixed positions 8*(15-m), so
    one compiled program serves all cores; the reversed-segment order
    makes the symmetric staircase consume psiA as a growing prefix.
  - on-chip: Chebyshev recurrence builds PHI [128, 65, 128] (VectorE +
    GpSimd split by position range), PE transposes + sqrt(softmax(w))
    scaling build psiA = weighted psi^T [65, 16384] (f32r).
  - row tile m (global row block 8m+c) multiplies its psi block against
    the psiA prefix [0, 1024*(16-m)) with K=65 f32r matmuls; PSUM f32
    results are evicted to fp16 strips (VectorE/ScalarE, load-balanced)
    and DMA'd out (alternating the two HWDGE rings).  fp16 output halves
    HBM write traffic; |G| <= 1 so the rounding error is <= 2^-11.
  - host: upcast to f32, scatter the segments back to global column
    order, mirror the uncomputed lower blocks from G[i,j] = G[j,i].
"""

import sys

if "/opt/trn_rl_repo" not in sys.path:
    sys.path.insert(0, "/opt/trn_rl_repo")

import numpy as np

N_PTS = 16384
MAX_N = 32
N_FEAT = 2 * MAX_N + 1  # 65
N_CORES = 8
ROWS_PER_CORE = N_PTS // N_CORES  # 2048
N_BLOCKS = N_PTS // 128  # 128 point-blocks
N_ROW_TILES = 16  # row tiles per core

# matmul operand dtype for psiA: "bf16" (fast LDWEIGHTS, half SBUF, adds
# ~2^-9-relative operand rounding), "f32r" (full-rate fp32, hw rounding).
MM_DTYPE = "bf16"

_CACHE = {}


def _block_order(c):
    """Host-side row order of xs_all for core c: position k holds global
    block (120 - 8*(k//8) + k%8 + c) mod 128."""
    return [(120 - 8 * (k // 8) + (k % 8) + c) % 128 for k in range(N_BLOCKS)]


def _build_nc():
    import concourse.bacc as bacc
    import concourse.tile as tile
    from concourse import mybir
    from concourse.masks import make_identity
    from contextlib import ExitStack

    f32 = mybir.dt.float32
    f16 = mybir.dt.float16
    mm_dt = {"bf16": mybir.dt.bfloat16,
             "f32r": mybir.dt.float32r,
             "f32": mybir.dt.float32}[MM_DTYPE]
    Act = mybir.ActivationFunctionType
    Alu = mybir.AluOpType

    nc = bacc.Bacc("TRN2", target_bir_lowering=False, debug=False,
                   num_devices=N_CORES)

    xs_all = nc.dram_tensor("xs_all", [128, 128], f32,
                            kind="ExternalInput").ap()
    logits = nc.dram_tensor("logits", [1, MAX_N + 1], f32,
                            kind="ExternalInput").ap()
    g = nc.dram_tensor("g", [ROWS_PER_CORE, N_PTS], f16,
                       kind="ExternalOutput").ap()

    with tile.TileContext(nc) as tc, ExitStack() as ctx:
        consts = ctx.enter_context(tc.tile_pool(name="consts", bufs=1))
        smalls = ctx.enter_context(tc.tile_pool(name="smalls", bufs=1))
        tmpp = ctx.enter_context(tc.tile_pool(name="tmpp", bufs=2))
        phip = ctx.enter_context(tc.tile_pool(name="phip", bufs=1))
        psip = ctx.enter_context(tc.tile_pool(name="psip", bufs=1))
        outp = ctx.enter_context(tc.tile_pool(name="outp", bufs=4))
        pre_ps = ctx.enter_context(
            tc.tile_pool(name="pre_ps", bufs=2, space="PSUM"))
        mm_ps = ctx.enter_context(
            tc.tile_pool(name="mm_ps", bufs=3, space="PSUM"))

        # ---- input DMAs (gpsimd ring: its preamble finishes ~1.3us
        # before sync's, so the recurrence starts earlier) ----------------
        X = smalls.tile([128, 128], f32, tag="X")
        nc.gpsimd.dma_start(X[:], xs_all[:])
        Lg = smalls.tile([1, MAX_N + 1], f32, tag="Lg")
        nc.gpsimd.dma_start(Lg[:], logits[:])

        # ---- constants --------------------------------------------------
        identity = consts.tile([128, 128], f32, tag="identity")
        make_identity(nc, identity[:])
        identity_bf = consts.tile([128, 128], mm_dt, tag="identity_bf")
        make_identity(nc, identity_bf[:])
        # dup[j, k] = 1 iff k == 2j or k == 2j-1 (degree-duplication map)
        dup = consts.tile([MAX_N + 1, N_FEAT], f32, tag="dup")
        nc.gpsimd.memset(dup[:], 0.0)
        nc.gpsimd.affine_select(
            out=dup[:], in_=dup[:], compare_op=Alu.not_equal, fill=1.0,
            base=0, pattern=[[-1, N_FEAT]], channel_multiplier=2)
        nc.gpsimd.affine_select(
            out=dup[:], in_=dup[:], compare_op=Alu.not_equal, fill=1.0,
            base=-1, pattern=[[-1, N_FEAT]], channel_multiplier=2)

        # ---- x transposed into point-block columns ----------------------
        XtF = smalls.tile([128, N_BLOCKS], f32, tag="XtF")
        xt_ps = pre_ps.tile([128, 128], f32, tag="pre")
        nc.tensor.transpose(xt_ps[:], X[:], identity[:])
        nc.vector.tensor_copy(XtF[:], xt_ps[:])

        # ---- softmax(logits) -> sqrt weights, expanded per feature -----
        SW65 = smalls.tile([N_FEAT, 1], f32, tag="SW65")

        def softmax_weights():
            E = smalls.tile([1, MAX_N + 1], f32, tag="E")
            nc.scalar.activation(E[:], Lg[:], Act.Exp)
            S = smalls.tile([1, 1], f32, tag="S")
            nc.vector.tensor_reduce(S[:], E[:], axis=mybir.AxisListType.X,
                                    op=Alu.add)
            R = smalls.tile([1, 1], f32, tag="R")
            nc.vector.reciprocal(R[:], S[:])
            W = smalls.tile([1, MAX_N + 1], f32, tag="W")
            nc.vector.tensor_scalar_mul(W[:], E[:], R[:])
            SW = smalls.tile([1, MAX_N + 1], f32, tag="SW")
            nc.scalar.activation(SW[:], W[:], Act.Sqrt)
            # (1, 33) -> (33, 1) via PE transpose, then expand to (65, 1)
            swc_ps = pre_ps.tile([MAX_N + 1, 1], f32, tag="pre")
            nc.tensor.transpose(swc_ps[:], SW[:], identity[0:1, 0:1])
            SWc = smalls.tile([MAX_N + 1, 1], f32, tag="SWc")
            nc.vector.tensor_copy(SWc[:], swc_ps[:])
            sw65_ps = pre_ps.tile([N_FEAT, 1], f32, tag="pre")
            nc.tensor.matmul(sw65_ps[:], dup[:], SWc[:], start=True,
                             stop=True)
            nc.vector.tensor_copy(SW65[:], sw65_ps[:])

        # ---- Chebyshev recurrence (features in PHI, position layout) ----
        # feature order: 0 -> 1;  2n-1 -> T_n;  2n -> s*U_{n-1} = sin(n@).
        # Degrees 1..8 come from the standard stride-1 pair recurrence;
        # degrees 9..32 use the stride-4 identity
        #   T_n = 2*T_4*T_{n-4} - T_{n-8},  sin(n@) likewise,
        # which splits into two independent chains (k=9,13,..,29 and
        # k=11,15,..,31) so VectorE and GpSimd recur in parallel with
        # 4-feature-wide ops.
        CW = 32  # recurrence chunk width (positions)
        x2 = smalls.tile([128, N_BLOCKS], f32, tag="x2")
        x2d2 = smalls.tile([128, 2, N_BLOCKS], f32, tag="x2d2")
        t4d4 = smalls.tile([128, 4, N_BLOCKS], f32, tag="t4d4")
        # per-chunk PHI tiles so the f32 -> bf16 conversion for the PE
        # transposes is ONE contiguous copy on GpSimd (a transpose with
        # fresh f32 weights cannot hide its LDWEIGHTS; 16-bit halves both
        # the weight load and the stream)
        PHIc = [phip.tile([128, N_FEAT, CW], f32, tag=f"PHI{ci}",
                          name=f"PHI{ci}")
                for ci in range(N_BLOCKS // CW)]
        PHIBc = [phip.tile([128, N_FEAT, CW], mm_dt, tag=f"PHIB{ci}",
                           name=f"PHIB{ci}")
                 for ci in range(N_BLOCKS // CW)]
        psiA = psip.tile([N_FEAT, N_BLOCKS * 128], mm_dt, tag="psiA")

        def rec_setup(E, ci):
            # degrees 1..8 plus the duplicated 2*T_4 multiplier
            c0, c1 = CW * ci, CW * (ci + 1)
            PHI = PHIc[ci]
            E.tensor_mul(x2[:, c0:c1], XtF[:, c0:c1], XtF[:, c0:c1])
            E.tensor_scalar_mul(x2d2[:, 0, c0:c1], XtF[:, c0:c1], 2.0)
            E.tensor_scalar_mul(x2d2[:, 1, c0:c1], XtF[:, c0:c1], 2.0)
            E.memset(PHI[:, 0, :], 1.0)
            E.tensor_copy(PHI[:, 1, :], XtF[:, c0:c1])  # T_1
            # s = sqrt(1 - x^2)  (|x| <= 1 so the argument >= 0 in fp32)
            nc.scalar.activation(PHI[:, 2, :], x2[:, c0:c1], Act.Sqrt,
                                 bias=1.0, scale=-1.0)       # s*U_0 = s
            E.tensor_scalar(PHI[:, 3, :], x2[:, c0:c1], 2.0,
                            -1.0, op0=Alu.mult, op1=Alu.add)  # T_2
            E.tensor_mul(PHI[:, 4, :], x2d2[:, 0, c0:c1],
                         PHI[:, 2, :])                       # s*U_1 = 2x*s
            for n in range(3, 9):
                tmp = tmpp.tile([128, 2, CW], f32, tag="tmpS")
                E.tensor_mul(tmp[:], PHI[:, 2 * n - 3:2 * n - 1, :],
                             x2d2[:, :, c0:c1])
                E.tensor_sub(PHI[:, 2 * n - 1:2 * n + 1, :], tmp[:],
                             PHI[:, 2 * n - 5:2 * n - 3, :])
            for j in range(4):
                E.tensor_scalar_mul(t4d4[:, j, c0:c1], PHI[:, 7, :], 2.0)

        def rec_chain(E, ks, tag, ci):
            # degrees {k, k+1} for k in ks: features [2k-1, 2k+3) =
            # 2*T_4 * [2k-9, 2k-5) - [2k-17, 2k-13)
            c0, c1 = CW * ci, CW * (ci + 1)
            PHI = PHIc[ci]
            for k in ks:
                tmp = tmpp.tile([128, 4, CW], f32, tag=tag)
                E.tensor_mul(tmp[:], PHI[:, 2 * k - 9:2 * k - 5, :],
                             t4d4[:, :, c0:c1])
                E.tensor_sub(PHI[:, 2 * k - 1:2 * k + 3, :], tmp[:],
                             PHI[:, 2 * k - 17:2 * k - 13, :])

        # ---- eviction load balancer (projected busy ns per engine) ------
        bal = {"v": 4000.0, "s": 3000.0}
        COST_V = 1224.0
        COST_S = 1114.0

        def evict_unit(dst, src):
            if bal["v"] + COST_V <= bal["s"] + COST_S:
                bal["v"] += COST_V
                nc.vector.tensor_copy(dst, src)
            else:
                bal["s"] += COST_S
                nc.scalar.copy(dst, src)

        # ---- psi^T blocks: PE transpose + sqrt(w) scaling ---------------
        def transposes(b0, b1):
            # groups of 4 share one PSUM bank and one scaled eviction
            # (bf16 PSUM source reads at 2x on VectorE)
            b = b0
            while b < b1:
                g_ = min(4, b1 - b)
                tps = pre_ps.tile([N_FEAT, g_ * 128], mm_dt, tag="pre")
                for i in range(g_):
                    bb = b + i
                    nc.tensor.transpose(tps[:, i * 128:(i + 1) * 128],
                                        PHIBc[bb // CW][:, :, bb % CW],
                                        identity_bf[:])
                dst = psiA[:, b * 128:(b + g_) * 128]
                if bal["v"] + 450.0 <= bal["s"] + 780.0:
                    bal["v"] += 450.0
                    nc.vector.tensor_scalar_mul(dst, tps[:], SW65[:])
                else:
                    bal["s"] += 780.0
                    nc.scalar.mul(dst, tps[:], SW65[:])
                b += g_

        # output DMAs: all on the Sync HWDGE ring (idle engine, no
        # head-of-line blocking; one ring row spans all 16 SDMA engines so
        # it is not a bandwidth limit)
        def strip_dma(dst, strip):
            nc.sync.dma_start(dst, strip)

        def gemm_m(m):
            # row tile m (global row block 8m+c): psiA prefix of
            # 1024*(16-m) columns, fp16 strips of up to 4096 columns.
            lhsT = psiA[:, 1024 * (15 - m):1024 * (15 - m) + 128]
            n_units = N_ROW_TILES - m
            u = 0
            while u < n_units:
                wu = min(4, n_units - u)
                strip = outp.tile([128, wu * 1024], f16, tag="strip")
                for j in range(wu):
                    c0 = (u + j) * 1024
                    ps = mm_ps.tile([128, 1024], f32, tag="ps")
                    nc.tensor.matmul(ps[:, 0:512], lhsT,
                                     psiA[:, c0:c0 + 512],
                                     start=True, stop=True)
                    nc.tensor.matmul(ps[:, 512:1024], lhsT,
                                     psiA[:, c0 + 512:c0 + 1024],
                                     start=True, stop=True)
                    evict_unit(strip[:, j * 1024:(j + 1) * 1024], ps[:])
                strip_dma(
                    g[m * 128:(m + 1) * 128, u * 1024:(u + wu) * 1024],
                    strip[:])
                u += wu

        # ---- pipelined emission -----------------------------------------
        # four recurrence chunks of 32 positions; position range [8s, 8s+8)
        # unlocks gemm_m(15-s).
        softmax_weights()
        CH_A = (9, 13, 17, 21, 25, 29)
        CH_B = (11, 15, 19, 23, 27, 31)
        for ci in range(4):
            rec_setup(nc.vector, ci)
            bal["v"] += 4700.0
            if ci == 0:
                # first chunk is latency-critical: A-chain on VectorE
                rec_chain(nc.vector, CH_A, "tmpA", ci)
                bal["v"] += 6400.0
            else:
                rec_chain(nc.gpsimd, CH_A, "tmpC", ci)
            rec_chain(nc.gpsimd, CH_B, "tmpB", ci)
            # contiguous 1D f32 -> bf16 conversion (~2.3us on VectorE;
            # GpSimd casts are 3x slower and would serialize the chunks)
            nc.vector.tensor_copy(PHIBc[ci][:], PHIc[ci][:])
            bal["v"] += 2300.0
        for ci in range(4):
            # all 32 transposes of the chunk in one PE burst (longer
            # gap-free PE runs), then its four gemms
            transposes(32 * ci, 32 * (ci + 1))
            for s in range(4 * ci, 4 * ci + 4):
                gemm_m(15 - s)

    nc.compile()
    return nc


def _get_nc():
    if "nc" not in _CACHE:
        _CACHE["nc"] = _build_nc()
    return _CACHE["nc"]


def _make_in_maps(xs, logits):
    xs = np.ascontiguousarray(np.asarray(xs, dtype=np.float32).reshape(N_PTS))
    lg = np.ascontiguousarray(
        np.asarray(logits, dtype=np.float32).reshape(1, MAX_N + 1))
    xa = xs.reshape(N_BLOCKS, 128)
    in_maps = []
    for c in range(N_CORES):
        in_maps.append({
            "xs_all": np.ascontiguousarray(xa[_block_order(c)]),
            "logits": lg,
        })
    return in_maps


def run(xs, logits, trace=False, tmpdir=None):
    """Run the SPMD kernel; returns (full output, BassKernelResults)."""
    from concourse.bass_utils import run_bass_kernel_spmd

    nc = _get_nc()
    in_maps = _make_in_maps(xs, logits)
    res = run_bass_kernel_spmd(nc, in_maps, list(range(N_CORES)),
                               trace=trace, tmpdir=tmpdir)
    # scatter computed strips to global column order, then mirror the
    # remaining lower blocks (G[i,j] = G[j,i], bit-exact on device)
    out = np.empty((N_PTS, N_PTS), np.float32)
    for c in range(N_CORES):
        gc = res.results[c]["g"]
        for m in range(N_ROW_TILES):
            A = 8 * m + c
            r0 = 128 * A
            S = gc[128 * m:128 * (m + 1),
                   0:1024 * (N_ROW_TILES - m)].astype(np.float32)
            for s in range(N_ROW_TILES - m):
                g0 = 120 - 8 * s + c
                seg = S[:, 1024 * s:1024 * (s + 1)]
                if g0 + 8 <= N_BLOCKS:
                    out[r0:r0 + 128, 128 * g0:128 * (g0 + 8)] = seg
                else:
                    k1 = N_BLOCKS - g0
                    out[r0:r0 + 128, 128 * g0:] = seg[:, :128 * k1]
                    out[r0:r0 + 128, :128 * (8 - k1)] = seg[:, 128 * k1:]
    for A in range(N_BLOCKS):
        c = A % 8
        r0 = 128 * A
        if A > c:
            out[r0:r0 + 128, 128 * c:128 * A] = \
                out[128 * c:128 * A, r0:r0 + 128].T
    return out, res


def kernel(xs, logits):
    out, _ = run(xs, logits, trace=False)
    return out


# revision 23
# speedup vs baseline: 1.0449x; 1.0449x over previous
"""Trainium2 Bass kernel for nn_ChebKernelMixture.

Computes gram(xs) = psi(xs) @ psi(xs).T where psi is a Chebyshev feature
map: psi(x) = concat_n sqrt(w_n) * phi_n(x), phi_0 = [1],
phi_n = [T_n(x), sqrt(1-x^2) U_{n-1}(x)], w = softmax(logits).

Shapes: xs (16384,), logits (33,) -> out (16384, 16384) f32.

Strategy (8 NeuronCores, SPMD, no collectives):
  - the host hands each core the full xs as a [128, 128] matrix whose
    rows are the 128 point-blocks in a core-specific order: position k
    holds global block (120 - 8*(k//8) + k%8 + c) mod 128.  The rotation
    by c puts the core's own row blocks at fixed positions 8*(15-m), so
    one compiled program serves all cores; the reversed-segment order
    makes the symmetric staircase consume psiA as a growing prefix.
  - on-chip: Chebyshev recurrence builds PHI [128, 65, 128] (VectorE +
    GpSimd split by position range), PE transposes + sqrt(softmax(w))
    scaling build psiA = weighted psi^T [65, 16384] (f32r).
  - row tile m (global row block 8m+c) multiplies its psi block against
    the psiA prefix [0, 1024*(16-m)) with K=65 f32r matmuls; PSUM f32
    results are evicted to fp16 strips (VectorE/ScalarE, load-balanced)
    and DMA'd out (alternating the two HWDGE rings).  fp16 output halves
    HBM write traffic; |G| <= 1 so the rounding error is <= 2^-11.
  - host: upcast to f32, scatter the segments back to global column
    order, mirror the uncomputed lower blocks from G[i,j] = G[j,i].
"""

import sys

if "/opt/trn_rl_repo" not in sys.path:
    sys.path.insert(0, "/opt/trn_rl_repo")

import numpy as np

N_PTS = 16384
MAX_N = 32
N_FEAT = 2 * MAX_N + 1  # 65
N_CORES = 8
ROWS_PER_CORE = N_PTS // N_CORES  # 2048
N_BLOCKS = N_PTS // 128  # 128 point-blocks
N_ROW_TILES = 16  # row tiles per core

# matmul operand dtype for psiA: "bf16" (fast LDWEIGHTS, half SBUF, adds
# ~2^-9-relative operand rounding), "f32r" (full-rate fp32, hw rounding).
MM_DTYPE = "bf16"

_CACHE = {}


def _block_order(c):
    """Host-side row order of xs_all for core c: position k holds global
    block (120 - 8*(k//8) + k%8 + c) mod 128."""
    return [(120 - 8 * (k // 8) + (k % 8) + c) % 128 for k in range(N_BLOCKS)]


def _build_nc():
    import concourse.bacc as bacc
    import concourse.tile as tile
    from concourse import mybir
    from concourse.masks import make_identity
    from contextlib import ExitStack

    f32 = mybir.dt.float32
    f16 = mybir.dt.float16
    mm_dt = {"bf16": mybir.dt.bfloat16,
             "f32r": mybir.dt.float32r,
             "f32": mybir.dt.float32}[MM_DTYPE]
    Act = mybir.ActivationFunctionType
    Alu = mybir.AluOpType

    nc = bacc.Bacc("TRN2", target_bir_lowering=False, debug=False,
                   num_devices=N_CORES)

    xs_all = nc.dram_tensor("xs_all", [128, 128], f32,
                            kind="ExternalInput").ap()
    logits = nc.dram_tensor("logits", [1, MAX_N + 1], f32,
                            kind="ExternalInput").ap()
    g = nc.dram_tensor("g", [ROWS_PER_CORE, N_PTS], f16,
                       kind="ExternalOutput").ap()

    with tile.TileContext(nc) as tc, ExitStack() as ctx:
        consts = ctx.enter_context(tc.tile_pool(name="consts", bufs=1))
        smalls = ctx.enter_context(tc.tile_pool(name="smalls", bufs=1))
        tmpp = ctx.enter_context(tc.tile_pool(name="tmpp", bufs=2))
        phip = ctx.enter_context(tc.tile_pool(name="phip", bufs=1))
        psip = ctx.enter_context(tc.tile_pool(name="psip", bufs=1))
        outp = ctx.enter_context(tc.tile_pool(name="outp", bufs=4))
        pre_ps = ctx.enter_context(
            tc.tile_pool(name="pre_ps", bufs=2, space="PSUM"))
        mm_ps = ctx.enter_context(
            tc.tile_pool(name="mm_ps", bufs=3, space="PSUM"))

        # ---- input DMAs (gpsimd ring: its preamble finishes ~1.3us
        # before sync's, so the recurrence starts earlier) ----------------
        X = smalls.tile([128, 128], f32, tag="X")
        nc.gpsimd.dma_start(X[:], xs_all[:])
        Lg = smalls.tile([1, MAX_N + 1], f32, tag="Lg")
        nc.gpsimd.dma_start(Lg[:], logits[:])

        # ---- constants --------------------------------------------------
        identity = consts.tile([128, 128], f32, tag="identity")
        make_identity(nc, identity[:])
        identity_bf = consts.tile([128, 128], mm_dt, tag="identity_bf")
        make_identity(nc, identity_bf[:])
        # dup[j, k] = 1 iff k == 2j or k == 2j-1 (degree-duplication map)
        dup = consts.tile([MAX_N + 1, N_FEAT], f32, tag="dup")
        nc.gpsimd.memset(dup[:], 0.0)
        nc.gpsimd.affine_select(
            out=dup[:], in_=dup[:], compare_op=Alu.not_equal, fill=1.0,
            base=0, pattern=[[-1, N_FEAT]], channel_multiplier=2)
        nc.gpsimd.affine_select(
            out=dup[:], in_=dup[:], compare_op=Alu.not_equal, fill=1.0,
            base=-1, pattern=[[-1, N_FEAT]], channel_multiplier=2)

        # ---- x transposed into point-block columns ----------------------
        XtF = smalls.tile([128, N_BLOCKS], f32, tag="XtF")
        xt_ps = pre_ps.tile([128, 128], f32, tag="pre")
        nc.tensor.transpose(xt_ps[:], X[:], identity[:])
        nc.vector.tensor_copy(XtF[:], xt_ps[:])

        # ---- softmax(logits) -> sqrt weights, expanded per feature -----
        SW65 = smalls.tile([N_FEAT, 1], f32, tag="SW65")

        def softmax_weights():
            E = smalls.tile([1, MAX_N + 1], f32, tag="E")
            nc.scalar.activation(E[:], Lg[:], Act.Exp)
            S = smalls.tile([1, 1], f32, tag="S")
            nc.vector.tensor_reduce(S[:], E[:], axis=mybir.AxisListType.X,
                                    op=Alu.add)
            R = smalls.tile([1, 1], f32, tag="R")
            nc.vector.reciprocal(R[:], S[:])
            W = smalls.tile([1, MAX_N + 1], f32, tag="W")
            nc.vector.tensor_scalar_mul(W[:], E[:], R[:])
            SW = smalls.tile([1, MAX_N + 1], f32, tag="SW")
            nc.scalar.activation(SW[:], W[:], Act.Sqrt)
            # (1, 33) -> (33, 1) via PE transpose, then expand to (65, 1)
            swc_ps = pre_ps.tile([MAX_N + 1, 1], f32, tag="pre")
            nc.tensor.transpose(swc_ps[:], SW[:], identity[0:1, 0:1])
            SWc = smalls.tile([MAX_N + 1, 1], f32, tag="SWc")
            nc.vector.tensor_copy(SWc[:], swc_ps[:])
            sw65_ps = pre_ps.tile([N_FEAT, 1], f32, tag="pre")
            nc.tensor.matmul(sw65_ps[:], dup[:], SWc[:], start=True,
                             stop=True)
            nc.vector.tensor_copy(SW65[:], sw65_ps[:])

        # ---- Chebyshev recurrence (features in PHI, position layout) ----
        # feature order: 0 -> 1;  2n-1 -> T_n;  2n -> s*U_{n-1} = sin(n@).
        # Degrees 1..8 come from the standard stride-1 pair recurrence;
        # degrees 9..32 use the stride-4 identity
        #   T_n = 2*T_4*T_{n-4} - T_{n-8},  sin(n@) likewise,
        # which splits into two independent chains (k=9,13,..,29 and
        # k=11,15,..,31) so VectorE and GpSimd recur in parallel with
        # 4-feature-wide ops.
        CW = 32  # recurrence chunk width (positions)
        x2 = smalls.tile([128, N_BLOCKS], f32, tag="x2")
        x2d2 = smalls.tile([128, 2, N_BLOCKS], f32, tag="x2d2")
        t4d4 = smalls.tile([128, 4, N_BLOCKS], f32, tag="t4d4")
        # per-chunk PHI tiles so the f32 -> bf16 conversion for the PE
        # transposes is ONE contiguous copy on GpSimd (a transpose with
        # fresh f32 weights cannot hide its LDWEIGHTS; 16-bit halves both
        # the weight load and the stream)
        PHIc = [phip.tile([128, N_FEAT, CW], f32, tag=f"PHI{ci}",
                          name=f"PHI{ci}")
                for ci in range(N_BLOCKS // CW)]
        PHIBc = [phip.tile([128, N_FEAT, CW], mm_dt, tag=f"PHIB{ci}",
                           name=f"PHIB{ci}")
                 for ci in range(N_BLOCKS // CW)]
        psiA = psip.tile([N_FEAT, N_BLOCKS * 128], mm_dt, tag="psiA")

        def rec_setup(E, ci):
            # degrees 1..8 plus the duplicated 2*T_4 multiplier
            c0, c1 = CW * ci, CW * (ci + 1)
            PHI = PHIc[ci]
            E.tensor_mul(x2[:, c0:c1], XtF[:, c0:c1], XtF[:, c0:c1])
            E.tensor_scalar_mul(x2d2[:, 0, c0:c1], XtF[:, c0:c1], 2.0)
            E.tensor_scalar_mul(x2d2[:, 1, c0:c1], XtF[:, c0:c1], 2.0)
            E.memset(PHI[:, 0, :], 1.0)
            E.tensor_copy(PHI[:, 1, :], XtF[:, c0:c1])  # T_1
            # s = sqrt(1 - x^2)  (|x| <= 1 so the argument >= 0 in fp32)
            nc.scalar.activation(PHI[:, 2, :], x2[:, c0:c1], Act.Sqrt,
                                 bias=1.0, scale=-1.0)       # s*U_0 = s
            E.tensor_scalar(PHI[:, 3, :], x2[:, c0:c1], 2.0,
                            -1.0, op0=Alu.mult, op1=Alu.add)  # T_2
            E.tensor_mul(PHI[:, 4, :], x2d2[:, 0, c0:c1],
                         PHI[:, 2, :])                       # s*U_1 = 2x*s
            for n in range(3, 9):
                tmp = tmpp.tile([128, 2, CW], f32, tag="tmpS")
                E.tensor_mul(tmp[:], PHI[:, 2 * n - 3:2 * n - 1, :],
                             x2d2[:, :, c0:c1])
                E.tensor_sub(PHI[:, 2 * n - 1:2 * n + 1, :], tmp[:],
                             PHI[:, 2 * n - 5:2 * n - 3, :])
            for j in range(4):
                E.tensor_scalar_mul(t4d4[:, j, c0:c1], PHI[:, 7, :], 2.0)

        def rec_chain(E, ks, tag, ci):
            # degrees {k, k+1} for k in ks: features [2k-1, 2k+3) =
            # 2*T_4 * [2k-9, 2k-5) - [2k-17, 2k-13)
            c0, c1 = CW * ci, CW * (ci + 1)
            PHI = PHIc[ci]
            for k in ks:
                tmp = tmpp.tile([128, 4, CW], f32, tag=tag)
                E.tensor_mul(tmp[:], PHI[:, 2 * k - 9:2 * k - 5, :],
                             t4d4[:, :, c0:c1])
                E.tensor_sub(PHI[:, 2 * k - 1:2 * k + 3, :], tmp[:],
                             PHI[:, 2 * k - 17:2 * k - 13, :])

        # ---- eviction load balancer (projected busy ns per engine) ------
        bal = {"v": 4000.0, "s": 3000.0}
        COST_V = 1224.0
        COST_S = 1114.0

        def evict_unit(dst, src):
            if bal["v"] + COST_V <= bal["s"] + COST_S:
                bal["v"] += COST_V
                nc.vector.tensor_copy(dst, src)
            else:
                bal["s"] += COST_S
                nc.scalar.copy(dst, src)

        # ---- psi^T blocks: PE transpose + sqrt(w) scaling ---------------
        def transposes(b0, b1):
            # groups of 4 share one PSUM bank and one scaled eviction
            # (bf16 PSUM source reads at 2x on VectorE)
            b = b0
            while b < b1:
                g_ = min(4, b1 - b)
                tps = pre_ps.tile([N_FEAT, g_ * 128], mm_dt, tag="pre")
                for i in range(g_):
                    bb = b + i
                    nc.tensor.transpose(tps[:, i * 128:(i + 1) * 128],
                                        PHIBc[bb // CW][:, :, bb % CW],
                                        identity_bf[:])
                dst = psiA[:, b * 128:(b + g_) * 128]
                if bal["v"] + 450.0 <= bal["s"] + 780.0:
                    bal["v"] += 450.0
                    nc.vector.tensor_scalar_mul(dst, tps[:], SW65[:])
                else:
                    bal["s"] += 780.0
                    nc.scalar.mul(dst, tps[:], SW65[:])
                b += g_

        # output DMAs: all on the Sync HWDGE ring (idle engine, no
        # head-of-line blocking; one ring row spans all 16 SDMA engines so
        # it is not a bandwidth limit)
        def strip_dma(dst, strip):
            nc.sync.dma_start(dst, strip)

        def gemm_m(m):
            # row tile m (global row block 8m+c): psiA prefix of
            # 1024*(16-m) columns, fp16 strips of up to 4096 columns.
            lhsT = psiA[:, 1024 * (15 - m):1024 * (15 - m) + 128]
            n_units = N_ROW_TILES - m
            u = 0
            while u < n_units:
                wu = min(4, n_units - u)
                strip = outp.tile([128, wu * 1024], f16, tag="strip")
                for j in range(wu):
                    c0 = (u + j) * 1024
                    ps = mm_ps.tile([128, 1024], f32, tag="ps")
                    nc.tensor.matmul(ps[:, 0:512], lhsT,
                                     psiA[:, c0:c0 + 512],
                                     start=True, stop=True)
                    nc.tensor.matmul(ps[:, 512:1024], lhsT,
                                     psiA[:, c0 + 512:c0 + 1024],
                                     start=True, stop=True)
                    evict_unit(strip[:, j * 1024:(j + 1) * 1024], ps[:])
                strip_dma(
                    g[m * 128:(m + 1) * 128, u * 1024:(u + wu) * 1024],
                    strip[:])
                u += wu

        # ---- pipelined emission -----------------------------------------
        # four recurrence chunks of 32 positions; position range [8s, 8s+8)
        # unlocks gemm_m(15-s).
        softmax_weights()
        CH_A = (9, 13, 17, 21, 25, 29)
        CH_B = (11, 15, 19, 23, 27, 31)
        for ci in range(4):
            rec_setup(nc.vector, ci)
            bal["v"] += 4700.0
            if ci == 0:
                # first chunk is latency-critical: A-chain on VectorE
                rec_chain(nc.vector, CH_A, "tmpA", ci)
                bal["v"] += 6400.0
            else:
                rec_chain(nc.gpsimd, CH_A, "tmpC", ci)
            rec_chain(nc.gpsimd, CH_B, "tmpB", ci)
            # contiguous 1D f32 -> bf16 conversion (~2.3us on VectorE;
            # GpSimd casts are 3x slower and would serialize the chunks)
            nc.vector.tensor_copy(PHIBc[ci][:], PHIc[ci][:])
            bal["v"] += 2300.0
        for s in range(16):
            transposes(8 * s, 8 * (s + 1))
            gemm_m(15 - s)

    nc.compile()
    return nc


def _get_nc():
    if "nc" not in _CACHE:
        _CACHE["nc"] = _build_nc()
    return _CACHE["nc"]


def _make_in_maps(xs, logits):
    xs = np.ascontiguousarray(np.asarray(xs, dtype=np.float32).reshape(N_PTS))
    lg = np.ascontiguousarray(
        np.asarray(logits, dtype=np.float32).reshape(1, MAX_N + 1))
    xa = xs.reshape(N_BLOCKS, 128)
    in_maps = []
    for c in range(N_CORES):
        in_maps.append({
            "xs_all": np.ascontiguousarray(xa[_block_order(c)]),
            "logits": lg,
        })
    return in_maps


def run(xs, logits, trace=False, tmpdir=None):
    """Run the SPMD kernel; returns (full output, BassKernelResults)."""
    from concourse.bass_utils import run_bass_kernel_spmd

    nc = _get_nc()
    in_maps = _make_in_maps(xs, logits)
    res = run_bass_kernel_spmd(nc, in_maps, list(range(N_CORES)),
                               trace=trace, tmpdir=tmpdir)
    # scatter computed strips to global column order, then mirror the
    # remaining lower blocks (G[i,j] = G[j,i], bit-exact on device)
    out = np.empty((N_PTS, N_PTS), np.float32)
    for c in range(N_CORES):
        gc = res.results[c]["g"]
        for m in range(N_ROW_TILES):
            A = 8 * m + c
            r0 = 128 * A
            S = gc[128 * m:128 * (m + 1),
                   0:1024 * (N_ROW_TILES - m)].astype(np.float32)
            for s in range(N_ROW_TILES - m):
                g0 = 120 - 8 * s + c
                seg = S[:, 1024 * s:1024 * (s + 1)]
                if g0 + 8 <= N_BLOCKS:
                    out[r0:r0 + 128, 128 * g0:128 * (g0 + 8)] = seg
                else:
                    k1 = N_BLOCKS - g0
                    out[r0:r0 + 128, 128 * g0:] = seg[:, :128 * k1]
                    out[r0:r0 + 128, :128 * (8 - k1)] = seg[:, 128 * k1:]
    for A in range(N_BLOCKS):
        c = A % 8
        r0 = 128 * A
        if A > c:
            out[r0:r0 + 128, 128 * c:128 * A] = \
                out[128 * c:128 * A, r0:r0 + 128].T
    return out, res


def kernel(xs, logits):
    out, _ = run(xs, logits, trace=False)
    return out
